# revision 1
# baseline (speedup 1.0000x reference)
import sys
for _p in ('/opt/trn_rl_repo',):
    if _p not in sys.path:
        sys.path.insert(0, _p)

"""NLSGCRN cell Bass/Tile kernel for TRN2, batch-sharded SPMD over 8 cores.

Per-core shapes (b_loc = 4 batches):
  x [4,2000,32], state [4,2000,64], x_full [4,12,2000,48], emb [2000,16],
  pools gw/uw/gb/ub/gT/uT, out h [4,2000,64].

Strategy:
  A = exp(relu(emb emb^T)) (symmetric, bf16), d = rowsum, rinv = 1/d;
  s^k x = A-chunk matmuls with 1/d row-scale on PSUM eviction (natural
  [n, (b,c)] layout). Per-node grouped GEMM via the D-expansion:
  y[r,(d,o)] = Xg[r,ki] @ WP[ki,(d,o)], z[r,o] = bias + sum_d e[r,d]*y[r,(d,o)]
  with per-partition-scalar fused multiply-add (scalar_tensor_tensor) split
  DVE (direct from PSUM) / GPSIMD (after bf16 eviction by ACT).
  Window conv: xt = sum_t T[t] x_full[:,t] accumulated on GPSIMD, contracted
  with the same machinery (ki = 48).
"""

from contextlib import ExitStack

import concourse.bass as bass
import concourse.tile as tile
from concourse import mybir
from concourse._compat import with_exitstack

F32 = mybir.dt.float32
F32R = mybir.dt.float32r
BF16 = mybir.dt.bfloat16
AF = mybir.ActivationFunctionType
OP = mybir.AluOpType

B_LOC = 4
N = 2000
NCHUNK = 16           # ceil(2000/128)
NFULL = (NCHUNK - 1) * 128   # 1920
NPAD = NCHUNK * 128   # 2048
R = B_LOC * NPAD      # 8192 padded rows
NRC = R // 128        # 64 row-chunks
DIN, DOUT = 32, 64
CIN = 96
CW = 48
WLEN = 12
EMB = 16
K = 3

DVE_PER_8 = 5   # of every 8 row-chunks, this many d-reduce on DVE, rest GPSIMD


def nlen(nch):
    return 128 if nch < NCHUNK - 1 else N - NFULL  # last = 80


def chunked_load(nc, dst, src, eng=None):
    """dst [128, NCHUNK, ...inner] <- src [2000, ...inner] splitting rows."""
    eng = eng or nc.sync
    inner = src.shape[1:]
    eng.dma_start(
        dst[:, 0 : NCHUNK - 1],
        src[0:NFULL].rearrange(
            "(c p) " + " ".join(f"i{j}" for j in range(len(inner)))
            + " -> p c " + " ".join(f"i{j}" for j in range(len(inner))),
            p=128,
        ),
    )
    eng.dma_start(dst[0 : N - NFULL, NCHUNK - 1], src[NFULL:N])


@with_exitstack
def build(ctx: ExitStack, tc: tile.TileContext, io: dict):
    nc = tc.nc

    io = {k: (v[:] if not isinstance(v, bass.AP) else v) for k, v in io.items()}
    x, state, x_full = io["x"], io["state"], io["x_full"]
    emb = io["node_embeddings"]
    out = io["out"]

    const = ctx.enter_context(tc.tile_pool(name="const", bufs=1))
    big = ctx.enter_context(tc.tile_pool(name="big", bufs=1))
    stage = ctx.enter_context(tc.tile_pool(name="stage", bufs=3))
    stage3 = ctx.enter_context(tc.tile_pool(name="stage3", bufs=2))

    # ================= constants / weights =================
    eexp = const.tile([128, NCHUNK, EMB], BF16)
    nc.vector.memset(eexp[:], 0.0)
    chunked_load(nc, eexp, emb, eng=nc.gpsimd)

    # WPg [128, 3, 1024] bf16: rows 0:96 = c, cols (d,o) d-major.
    # Pad rows of the weight tiles can be anything on HW (the matching XgT
    # partitions are zero), but memset keeps the simulator's init-tracking
    # happy.
    WPg = const.tile([128, K, EMB * 64], BF16)
    nc.vector.memset(WPg[:], 0.0)
    WPu = const.tile([128, K, EMB * 32], BF16)
    nc.vector.memset(WPu[:], 0.0)
    WWg = const.tile([128, EMB * 64], BF16)
    nc.vector.memset(WWg[:], 0.0)
    WWu = const.tile([128, EMB * 32], BF16)
    nc.vector.memset(WWu[:], 0.0)
    for k in range(K):
        wk = stage.tile([128, EMB, 64], F32, tag="stg")
        nc.sync.dma_start(wk[0:CIN], io["gw_pool"][:, k].rearrange("d c o -> c d o"))
        nc.vector.tensor_copy(
            WPg[0:CIN, k].rearrange("p (d o) -> p d o", d=EMB), wk[0:CIN]
        )
        wku = stage.tile([128, EMB, 32], F32, tag="stg")
        nc.sync.dma_start(wku[0:32], io["uw_pool"][:, k, 0:32, :].rearrange("d c o -> c d o"))
        nc.sync.dma_start(wku[64:128], io["uw_pool"][:, k, 32:96, :].rearrange("d c o -> c d o"))
        nc.vector.tensor_copy(
            WPu[0:32, k].rearrange("p (d o) -> p d o", d=EMB), wku[0:32]
        )
        nc.vector.tensor_copy(
            WPu[64:128, k].rearrange("p (d o) -> p d o", d=EMB), wku[64:128]
        )
    wg = stage.tile([128, EMB, 64], F32, tag="stg")
    nc.sync.dma_start(wg[0:CW], io["gw_win"].rearrange("d i o -> i d o"))
    nc.vector.tensor_copy(WWg[0:CW].rearrange("p (d o) -> p d o", d=EMB), wg[0:CW])
    wu = stage.tile([128, EMB, 32], F32, tag="stg")
    # rows 64:112 (matches packed XtT where xt_u.T sits at partitions 64:112)
    nc.sync.dma_start(wu[64 : 64 + CW], io["uw_win"].rearrange("d i o -> i d o"))
    nc.vector.tensor_copy(
        WWu[64 : 64 + CW].rearrange("p (d o) -> p d o", d=EMB), wu[64 : 64 + CW]
    )

    Tb = const.tile([128, 2, WLEN], F32)
    for w, name in ((0, "gT"), (1, "uT")):
        src = io[name][:]
        nc.sync.dma_start(
            Tb[:, w, :],
            bass.AP(tensor=src.tensor, offset=src.offset, ap=[[0, 128]] + list(src.ap)),
        )

    # ================= biases + A (pre phase PSUM) =================
    biasg = const.tile([128, NCHUNK, 2 * DOUT], BF16)
    biasu = const.tile([128, NCHUNK, DOUT], BF16)
    nc.vector.memset(biasg[64:, NCHUNK - 1], 0.0)
    nc.vector.memset(biasu[64:, NCHUNK - 1], 0.0)
    A = big.tile([128, NCHUNK, N], BF16, tag="A")
    rinv = const.tile([128, NCHUNK], F32)
    dsum_all = const.tile([128, NCHUNK], F32)

    with tc.tile_pool(name="prep", bufs=1) as prep:
        embT_raw = prep.tile([EMB, N], F32)
        nc.sync.dma_start(embT_raw[:], emb.rearrange("n d -> d n"))
        embT = prep.tile([EMB, N], F32R)
        nc.vector.tensor_copy(embT[:], embT_raw[:])
        gbp_raw = prep.tile([EMB, 2 * DOUT], F32)
        nc.sync.dma_start(gbp_raw[:], io["gb_pool"][:])
        gbp_s = prep.tile([EMB, 2 * DOUT], F32R)
        nc.vector.tensor_copy(gbp_s[:], gbp_raw[:])
        ubp_raw = prep.tile([EMB, DOUT], F32)
        nc.sync.dma_start(ubp_raw[:], io["ub_pool"][:])
        ubp_s = prep.tile([EMB, DOUT], F32R)
        nc.vector.tensor_copy(ubp_s[:], ubp_raw[:])
        with tc.tile_pool(name="psum_pre", bufs=2, space="PSUM") as psum_pre:
            for nch in range(NCHUNK):
                l = nlen(nch)
                nsl = slice(nch * 128, nch * 128 + l)
                pg = psum_pre.tile([128, N], F32, tag="pg")
                for mj in range(4):
                    m0 = mj * 512
                    mw = min(512, N - m0)
                    nc.tensor.matmul(
                        pg[:l, m0 : m0 + mw], embT[:, nsl],
                        embT[:, m0 : m0 + mw], start=True, stop=True,
                    )
                nc.scalar.activation(A[:l, nch, :], pg[:l, :], AF.Exp)
                nc.vector.tensor_scalar(
                    out=A[:l, nch, :], in0=A[:l, nch, :],
                    scalar1=1.0, scalar2=0.0, op0=OP.max, op1=OP.add,
                    accum_out=dsum_all[:l, nch : nch + 1],
                )
                nc.vector.reciprocal(rinv[:l, nch : nch + 1], dsum_all[:l, nch : nch + 1])
        with tc.tile_pool(name="psum_b", bufs=2, space="PSUM") as psum_b:
            for nch in range(NCHUNK):
                l = nlen(nch)
                nsl = slice(nch * 128, nch * 128 + l)
                pb = psum_b.tile([128, 3 * DOUT], F32, tag="pbias")
                nc.tensor.matmul(
                    pb[:l, 0 : 2 * DOUT], embT[:, nsl],
                    gbp_s[:], start=True, stop=True,
                )
                nc.tensor.matmul(
                    pb[:l, 2 * DOUT :], embT[:, nsl],
                    ubp_s[:], start=True, stop=True,
                )
                nc.scalar.copy(biasg[:l, nch, :], pb[:l, 0 : 2 * DOUT])
                nc.scalar.copy(biasu[:l, nch, :], pb[:l, 2 * DOUT :])

    # ================= x/state load; X1 [128, nch, b, 128] bf16 =================
    X1 = big.tile([128, NCHUNK, B_LOC, 128], BF16, tag="slot1")
    nc.vector.memset(X1[:], 0.0)
    for b in range(B_LOC):
        xs = stage.tile([128, NCHUNK, DIN], F32, tag="stg")
        nc.vector.memset(xs[64:, NCHUNK - 1], 0.0)
        chunked_load(nc, xs, x[b])
        nc.vector.tensor_copy(X1[:, :, b, 0:DIN], xs[:])
        ss = stage.tile([128, NCHUNK, DOUT], F32, tag="stg")
        nc.vector.memset(ss[64:, NCHUNK - 1], 0.0)
        chunked_load(nc, ss, state[b])
        nc.scalar.copy(X1[:, :, b, DIN:CIN], ss[:])

    # ================= window t-contraction (flat layout) =================
    # x_full[b, t] is accumulated in a flat [128, 750] view (2000*48 elems
    # row-major): elementwise sums don't care about layout, and flat DMAs are
    # fully contiguous. Results bounce through DRAM into packed [NPAD, 128]
    # transpose sources.
    dram = ctx.enter_context(tc.tile_pool(name="dram", bufs=6, space="DRAM"))
    HNCH = NCHUNK // 2   # half-panel: 8 n-chunks = 1024 rows

    def half_panel_T(SRC, b, h):
        """[128, 1024] <- transpose of SRC[:, h*8:(h+1)*8, b, :] via DRAM."""
        t = xgt_pool.tile([128, HNCH * 128], BF16, tag="xgt")
        dp = dram.tile([HNCH * 128, 128], BF16, tag="panh")
        nc.sync.dma_start(
            dp.rearrange("(c p) o -> p c o", p=128),
            SRC[:, h * HNCH : (h + 1) * HNCH, b, :],
        )
        nc.sync.dma_start(t[:], dp[:], transpose=True)
        return t

    zeros128 = const.tile([128, 128], BF16)
    nc.vector.memset(zeros128[:], 0.0)
    dzero = dram.tile([NPAD, 128], BF16, tag="dzero")
    nc.sync.dma_start(
        dzero.rearrange("(c p) o -> p c o", p=128),
        bass.AP(tensor=zeros128.tensor, offset=zeros128.offset,
                ap=[[1, 128], [0, NCHUNK], [1, 128]]),
    )

    FLAT = N * CW // 128  # 750
    xt_g = big.tile([128, B_LOC, FLAT], F32, tag="xt_g")
    xt_u = big.tile([128, B_LOC, FLAT], F32, tag="xt_u")
    for t in range(WLEN):
        for b in range(B_LOC):
            st = stage3.tile([128, FLAT], F32, tag="xw")
            nc.sync.dma_start(
                st[:],
                x_full[b, t].rearrange("n i -> (n i)").rearrange("(p f) -> p f", p=128),
            )
            for w, acc in ((0, xt_g), (1, xt_u)):
                eng = nc.vector
                if t == 0:
                    eng.tensor_scalar(
                        out=acc[:, b, :], in0=st[:],
                        scalar1=Tb[:, w, 0:1], scalar2=None, op0=OP.mult,
                    )
                else:
                    eng.scalar_tensor_tensor(
                        out=acc[:, b, :], in0=st[:],
                        scalar=Tb[:, w, t : t + 1],
                        in1=acc[:, b, :], op0=OP.mult, op1=OP.add,
                    )

    # pack via DRAM: XtT partitions 0:48 = xt_g.T, 64:112 = xt_u.T
    XtT = big.tile([128, R], BF16, tag="XtT")
    for b in range(B_LOC):
        xgb16 = stage.tile([128, 2, FLAT], BF16, tag="stg")
        nc.gpsimd.tensor_copy(xgb16[:, 0, :], xt_g[:, b, :])
        nc.gpsimd.tensor_copy(xgb16[:, 1, :], xt_u[:, b, :])
        dflat = dram.tile([2, 128, FLAT], BF16, tag="dflat")
        nc.sync.dma_start(dflat.rearrange("w p f -> p w f"), xgb16[:])
        dpan = dram.tile([NPAD, 128], BF16, tag="pan")
        dfv = dflat.rearrange("w p f -> w (p f)").rearrange("w (n i) -> w n i", n=N)
        nc.sync.dma_start(dpan[0:N, 0:CW], dfv[0])
        nc.sync.dma_start(dpan[0:N, 64 : 64 + CW], dfv[1])
        # zero-fill the gaps so the transposed pad partitions are defined
        nc.sync.dma_start(dpan[0:N, CW:64], dzero[0:N, 0:16])
        nc.sync.dma_start(dpan[0:N, 112:128], dzero[0:N, 0:16])
        nc.sync.dma_start(dpan[N:NPAD, :], dzero[N:NPAD, :])
        nc.sync.dma_start(XtT[:, b * NPAD : (b + 1) * NPAD], dpan[:], transpose=True)

    # ================= diffusion helper =================
    def diffuse(psum_pool, SRC, DST, c0, clen):
        for nch in range(NCHUNK):
            l = nlen(nch)
            ph = psum_pool.tile([128, B_LOC, clen], F32, tag="pdiff")
            for mi in range(NCHUNK):
                ml = nlen(mi)
                nc.tensor.matmul(
                    ph[:l], A[:ml, mi, nch * 128 : nch * 128 + l],
                    SRC[:ml, mi, :, c0 : c0 + clen],
                    start=(mi == 0), stop=(mi == NCHUNK - 1),
                )
            nc.scalar.activation(
                DST[:l, nch, :, c0 : c0 + clen], ph[:l],
                AF.Copy, scale=rinv[:l, nch : nch + 1],
            )

    # ================= gate diffusion =================
    X2 = big.tile([128, NCHUNK, B_LOC, 128], BF16, tag="slot2")
    X3 = big.tile([128, NCHUNK, B_LOC, 128], BF16, tag="slot3")
    nc.gpsimd.memset(X2[:], 0.0)
    nc.gpsimd.memset(X3[:], 0.0)
    with tc.tile_pool(name="psum_d1", bufs=3, space="PSUM") as psum_d1:
        diffuse(psum_d1, X1, X2, 0, CIN)
        diffuse(psum_d1, X2, X3, 0, CIN)

    # ================= shared y-phase machinery =================
    acc_pool = ctx.enter_context(tc.tile_pool(name="accp", bufs=3))
    stf_pool = ctx.enter_context(tc.tile_pool(name="stf", bufs=3))
    ybf_pool = ctx.enter_context(tc.tile_pool(name="ybf", bufs=2))
    xgt_pool = ctx.enter_context(tc.tile_pool(name="xgt", bufs=4))

    def dred(psum, nblk, owid, acc, nch, use_dve, bias):
        acc3 = acc.rearrange("p (blk o) -> p blk o", blk=nblk)
        src = ybf_pool.tile([128, nblk * EMB * owid], BF16, tag="ybf")
        nc.scalar.copy(src[:], psum[:])
        eng = nc.vector
        src3 = src.rearrange("p (blk rest) -> p blk rest", blk=nblk)
        bias3 = bias.rearrange("p (blk o) -> p blk o", blk=nblk)
        for d in range(EMB):
            eng.scalar_tensor_tensor(
                out=acc3[:],
                in0=src3[:, :, d * owid : (d + 1) * owid],
                scalar=eexp[:, nch, d : d + 1],
                in1=bias3[:] if d == 0 else acc3[:],
                op0=OP.mult, op1=OP.add,
            )


    # ================= gate y-GEMM + d-red + gating =================
    r_gate = big.tile([128, NCHUNK, B_LOC, DOUT], BF16, tag="xt_g")

    with tc.tile_pool(name="psum_yg", bufs=2, space="PSUM") as psum_yg:
        for b in range(B_LOC):
          for h in range(2):
            xgb = [half_panel_T(S, b, h) for S in (X1, X2, X3)]
            for nch2 in range(HNCH):
                nch = h * HNCH + nch2
                rc = b * NCHUNK + nch
                l = nlen(nch)
                r0 = b * NPAD + nch * 128
                py = psum_yg.tile([128, 2048], F32, tag="pyg")
                for half in range(2):
                    ps = py[:, half * 512 : (half + 1) * 512]
                    for k in range(K):
                        nc.tensor.matmul(
                            ps, xgb[k][:, nch2 * 128 : (nch2 + 1) * 128],
                            WPg[:, k, half * 512 : (half + 1) * 512],
                            start=(k == 0), stop=(k == K - 1),
                        )
                for half in range(2):
                    nc.tensor.matmul(
                        py[:, 1024 + half * 512 : 1024 + (half + 1) * 512],
                        XtT[:, r0 : r0 + 128],
                        WWg[:, half * 512 : (half + 1) * 512],
                        start=True, stop=True,
                    )
                acc = acc_pool.tile([128, 2 * DOUT], BF16, tag="accz")
                dred(py, 2, DOUT, acc, nch, use_dve=(rc % 8) < DVE_PER_8,
                     bias=biasg[:, nch, :])
                ztile = acc_pool.tile([128, DOUT], BF16, tag="ztile")
                nc.scalar.activation(ztile[:], acc[:, 0:DOUT], AF.Sigmoid)
                nc.scalar.activation(r_gate[:, nch, b, :], acc[:, DOUT:], AF.Sigmoid)
                # CAND panel reuses X1's slot: cols 0:32 keep x; stale cols
                # 32:64 are neutralized by WPu's zero rows; z*state -> 64:128.
                stf = stf_pool.tile([128, DOUT], F32, tag="stf")
                if l < 128:
                    nc.vector.memset(stf[64:], 0.0)
                nc.sync.dma_start(stf[:l], state[b, nch * 128 : nch * 128 + l, :])
                nc.gpsimd.tensor_mul(X1[:, nch, b, 64:128], ztile[:], stf[:])

    CAND = X1  # renamed: panels now hold [x | 0 | z*state]

    # ================= update diffusion =================
    # C2/C3 reuse X2/X3 slots: cols 0:32 already hold diffused-x hops; zero
    # 32:64; diffusion writes 64:128.
    C2, C3 = X2, X3
    with tc.tile_pool(name="psum_d2", bufs=3, space="PSUM") as psum_d2:
        diffuse(psum_d2, CAND, C2, 64, DOUT)
        diffuse(psum_d2, C2, C3, 64, DOUT)

    # ================= update y-GEMM + d-red + output =================
    with tc.tile_pool(name="psum_yu", bufs=3, space="PSUM") as psum_yu:
        for b in range(B_LOC):
          for h in range(2):
            xgb = [half_panel_T(S, b, h) for S in (CAND, C2, C3)]
            for nch2 in range(HNCH):
                nch = h * HNCH + nch2
                rc = b * NCHUNK + nch
                l = nlen(nch)
                r0 = b * NPAD + nch * 128
                pu = psum_yu.tile([128, 1024], F32, tag="pyu")
                for k in range(K):
                    nc.tensor.matmul(
                        pu[:, 0:512], xgb[k][:, nch2 * 128 : (nch2 + 1) * 128],
                        start=(k == 0), stop=(k == K - 1), rhs=WPu[:, k, :],
                    )
                nc.tensor.matmul(
                    pu[:, 512:1024], XtT[:, r0 : r0 + 128], WWu[:],
                    start=True, stop=True,
                )
                accu = acc_pool.tile([128, DOUT], BF16, tag="accu")
                dred(pu, 2, 32, accu, nch, use_dve=(rc % 8) < DVE_PER_8,
                     bias=biasu[:, nch, :])
                hc = acc_pool.tile([128, DOUT], F32, tag="hc")
                nc.scalar.activation(hc[:], accu[:], AF.Tanh)
                tmp = acc_pool.tile([128, DOUT], F32, tag="tmp")
                stf = stf_pool.tile([128, DOUT], F32, tag="stf")
                if l < 128:
                    nc.vector.memset(stf[64:], 0.0)
                nc.sync.dma_start(stf[:l], state[b, nch * 128 : nch * 128 + l, :])
                nc.gpsimd.tensor_sub(tmp[:], stf[:], hc[:])
                nc.gpsimd.tensor_mul(tmp[:], tmp[:], r_gate[:, nch, b, :])
                nc.gpsimd.tensor_add(tmp[:], tmp[:], hc[:])
                nc.sync.dma_start(out[b, nch * 128 : nch * 128 + l, :], tmp[:l, :])


def make_io(nc):
    io = {}
    io["x"] = nc.dram_tensor("x", [B_LOC, N, DIN], F32, kind="ExternalInput")
    io["state"] = nc.dram_tensor("state", [B_LOC, N, DOUT], F32, kind="ExternalInput")
    io["x_full"] = nc.dram_tensor("x_full", [B_LOC, WLEN, N, CW], F32, kind="ExternalInput")
    io["node_embeddings"] = nc.dram_tensor("node_embeddings", [N, EMB], F32, kind="ExternalInput")
    io["gw_pool"] = nc.dram_tensor("gw_pool", [EMB, K, CIN, 64], F32, kind="ExternalInput")
    io["gw_win"] = nc.dram_tensor("gw_win", [EMB, CW, 64], F32, kind="ExternalInput")
    io["gb_pool"] = nc.dram_tensor("gb_pool", [EMB, 2 * DOUT], F32, kind="ExternalInput")
    io["gT"] = nc.dram_tensor("gT", [WLEN], F32, kind="ExternalInput")
    io["uw_pool"] = nc.dram_tensor("uw_pool", [EMB, K, CIN, 32], F32, kind="ExternalInput")
    io["uw_win"] = nc.dram_tensor("uw_win", [EMB, CW, 32], F32, kind="ExternalInput")
    io["ub_pool"] = nc.dram_tensor("ub_pool", [EMB, DOUT], F32, kind="ExternalInput")
    io["uT"] = nc.dram_tensor("uT", [WLEN], F32, kind="ExternalInput")
    io["out"] = nc.dram_tensor("out", [B_LOC, N, DOUT], F32, kind="ExternalOutput")
    return io


def build_module(debug=False):
    from concourse import bacc

    nc = bacc.Bacc("TRN2", target_bir_lowering=False, debug=debug)
    io = make_io(nc)
    with tile.TileContext(nc) as tc:
        build(tc, io)
    nc.finalize()
    return nc


# ======================= harness wrapper =======================
import numpy as _np

N_CORES = 8
_CACHE = {}


def _get_module():
    if "nc" not in _CACHE:
        _CACHE["nc"] = build_module()
    return _CACHE["nc"]


def kernel(**inputs):
    """Full-input entry point: shards over batch across 8 NeuronCores."""
    nc = _get_module()
    from concourse.bass_utils import run_bass_kernel_spmd

    xb = _np.ascontiguousarray(inputs["x"], dtype=_np.float32)
    sb = _np.ascontiguousarray(inputs["state"], dtype=_np.float32)
    xf = _np.ascontiguousarray(inputs["x_full"], dtype=_np.float32)
    rep = {
        k: _np.ascontiguousarray(inputs[k], dtype=_np.float32)
        for k in ("node_embeddings", "gw_pool", "gw_win", "gb_pool", "gT",
                  "uw_pool", "uw_win", "ub_pool", "uT")
    }
    in_maps = []
    for i in range(N_CORES):
        m = dict(rep)
        m["x"] = xb[i * B_LOC : (i + 1) * B_LOC]
        m["state"] = sb[i * B_LOC : (i + 1) * B_LOC]
        m["x_full"] = xf[i * B_LOC : (i + 1) * B_LOC]
        in_maps.append(m)
    res = run_bass_kernel_spmd(nc, in_maps, core_ids=list(range(N_CORES)))
    return _np.concatenate([res.results[i]["out"] for i in range(N_CORES)], axis=0)



# revision 43
# speedup vs baseline: 1.1872x; 1.1872x over previous
import sys
for _p in ('/opt/trn_rl_repo',):
    if _p not in sys.path:
        sys.path.insert(0, _p)

"""NLSGCRN cell Bass/Tile kernel for TRN2, batch-sharded SPMD over 8 cores.

Per-core shapes (b_loc = 4 batches):
  x [4,2000,32], state [4,2000,64], x_full [4,12,2000,48], emb [2000,16],
  pools gw/uw/gb/ub/gT/uT, out h [4,2000,64].

Strategy:
  A = exp(relu(emb emb^T)) (symmetric, bf16), d = rowsum, rinv = 1/d;
  s^k x = A-chunk matmuls with 1/d row-scale on PSUM eviction (natural
  [n, (b,c)] layout). Per-node grouped GEMM via the D-expansion:
  y[r,(d,o)] = Xg[r,ki] @ WP[ki,(d,o)], z[r,o] = bias + sum_d e[r,d]*y[r,(d,o)]
  with per-partition-scalar fused multiply-add (scalar_tensor_tensor) split
  DVE (direct from PSUM) / GPSIMD (after bf16 eviction by ACT).
  Window conv: xt = sum_t T[t] x_full[:,t] accumulated on GPSIMD, contracted
  with the same machinery (ki = 48).
"""

from contextlib import ExitStack

import concourse.bass as bass
import concourse.tile as tile
from concourse import mybir
from concourse._compat import with_exitstack

F32 = mybir.dt.float32
F32R = mybir.dt.float32r
BF16 = mybir.dt.bfloat16
AF = mybir.ActivationFunctionType
OP = mybir.AluOpType

B_LOC = 4
N = 2000
NCHUNK = 16           # ceil(2000/128)
NFULL = (NCHUNK - 1) * 128   # 1920
NPAD = NCHUNK * 128   # 2048
R = B_LOC * NPAD      # 8192 padded rows
NRC = R // 128        # 64 row-chunks
DIN, DOUT = 32, 64
CIN = 96
CW = 48
WLEN = 12
EMB = 16
K = 3

FLAT = N * CW // 128  # 750 (flat per-partition window elements)
DEBUG = False


def nlen(nch):
    return 128 if nch < NCHUNK - 1 else N - NFULL  # last = 80


def chunked_load(nc, dst, src, eng=None):
    """dst [128, NCHUNK, ...inner] <- src [2000, ...inner] splitting rows."""
    eng = eng or nc.sync
    inner = src.shape[1:]
    eng.dma_start(
        dst[:, 0 : NCHUNK - 1],
        src[0:NFULL].rearrange(
            "(c p) " + " ".join(f"i{j}" for j in range(len(inner)))
            + " -> p c " + " ".join(f"i{j}" for j in range(len(inner))),
            p=128,
        ),
    )
    eng.dma_start(dst[0 : N - NFULL, NCHUNK - 1], src[NFULL:N])


@with_exitstack
def build(ctx: ExitStack, tc: tile.TileContext, io: dict):
    nc = tc.nc

    io = {k: (v[:] if not isinstance(v, bass.AP) else v) for k, v in io.items()}
    x, state, x_full = io["x"], io["state"], io["x_full"]
    emb = io["node_embeddings"]
    out = io["out"]

    const = ctx.enter_context(tc.tile_pool(name="const", bufs=1))
    big = ctx.enter_context(tc.tile_pool(name="big", bufs=1))
    stage_scope = tc.tile_pool(name="stage", bufs=3)
    stage = stage_scope.__enter__()
    stage3_scope = tc.tile_pool(name="stage3", bufs=2)
    stage3 = stage3_scope.__enter__()

    # ================= constants / weights =================
    eexp = const.tile([128, NCHUNK, EMB], F32)
    nc.vector.memset(eexp[:], 0.0)
    chunked_load(nc, eexp, emb)

    # WPg [128, 3, 1024] bf16: rows 0:96 = c, cols (d,o) d-major.
    # Pad rows of the weight tiles can be anything on HW (the matching XgT
    # partitions are zero), but memset keeps the simulator's init-tracking
    # happy.
    WPg = const.tile([128, K, EMB * 64], BF16)
    nc.vector.memset(WPg[:], 0.0)
    WPu = const.tile([128, K, EMB * 32], BF16)
    nc.vector.memset(WPu[:], 0.0)
    WWg = const.tile([128, EMB * 64], BF16)
    nc.vector.memset(WWg[:], 0.0)
    WWu = const.tile([128, EMB * 32], BF16)
    nc.vector.memset(WWu[:], 0.0)
    for k in range(K):
        wk = stage.tile([128, EMB, 64], F32, tag="stg")
        nc.sync.dma_start(wk[0:CIN], io["gw_pool"][:, k].rearrange("d c o -> c d o"))
        nc.vector.tensor_copy(
            WPg[0:CIN, k].rearrange("p (d o) -> p d o", d=EMB), wk[0:CIN]
        )
        wku = stage.tile([128, EMB, 32], F32, tag="stg")
        nc.sync.dma_start(wku[0:32], io["uw_pool"][:, k, 0:32, :].rearrange("d c o -> c d o"))
        nc.sync.dma_start(wku[64:128], io["uw_pool"][:, k, 32:96, :].rearrange("d c o -> c d o"))
        nc.vector.tensor_copy(
            WPu[0:32, k].rearrange("p (d o) -> p d o", d=EMB), wku[0:32]
        )
        nc.vector.tensor_copy(
            WPu[64:128, k].rearrange("p (d o) -> p d o", d=EMB), wku[64:128]
        )
    wg = stage.tile([128, EMB, 64], F32, tag="stg")
    nc.sync.dma_start(wg[0:CW], io["gw_win"].rearrange("d i o -> i d o"))
    nc.vector.tensor_copy(WWg[0:CW].rearrange("p (d o) -> p d o", d=EMB), wg[0:CW])
    wu = stage.tile([128, EMB, 32], F32, tag="stg")
    # rows 64:112 (matches packed XtT where xt_u.T sits at partitions 64:112)
    nc.sync.dma_start(wu[64 : 64 + CW], io["uw_win"].rearrange("d i o -> i d o"))
    nc.vector.tensor_copy(
        WWu[64 : 64 + CW].rearrange("p (d o) -> p d o", d=EMB), wu[64 : 64 + CW]
    )

    # Bias folding: ones-rows in the X panels / XtT meet these weight rows, so
    # PSUM accumulates emb@bias_pool without a separate bias pass.
    #   gate z bias -> WPg row 96 (X1 ones col 96, k=0)
    #   gate r bias -> WWg row 48 (XtT ones row 48)
    #   update graph bias -> WPu row 32 (CAND ones col 32, k=0)
    #   update window bias -> WWu row 48
    # gpsimd DMAs cast f32 -> bf16 in flight.
    def _row1(src2d):
        return bass.AP(tensor=src2d.tensor, offset=src2d.offset,
                       ap=[[0, 1]] + [list(d) for d in src2d.ap])

    nc.gpsimd.dma_start(
        WPg[96:97, 0, :].rearrange("p (d o) -> p d o", d=EMB),
        _row1(io["gb_pool"][:, 0:DOUT]),
    )
    nc.gpsimd.dma_start(
        WWg[48:49, :].rearrange("p (d o) -> p d o", d=EMB),
        _row1(io["gb_pool"][:, DOUT:]),
    )
    nc.gpsimd.dma_start(
        WPu[32:33, 0, :].rearrange("p (d o) -> p d o", d=EMB),
        _row1(io["ub_pool"][:, 0:32]),
    )
    nc.gpsimd.dma_start(
        WWu[48:49, :].rearrange("p (d o) -> p d o", d=EMB),
        _row1(io["ub_pool"][:, 32:64]),
    )

    Tb = const.tile([128, 2, WLEN], F32)
    for w, name in ((0, "gT"), (1, "uT")):
        src = io[name][:]
        nc.sync.dma_start(
            Tb[:, w, :],
            bass.AP(tensor=src.tensor, offset=src.offset, ap=[[0, 128]] + list(src.ap)),
        )

    # ================= window t-contraction on PE =================
    # xt[p, f] = sum_t T[t] * xf_t[p, f] as 12 accumulating matmuls with
    # stationary diag(T[t]) built from the host-provided identity. Runs first
    # so PE ramps up while embT/x/state DMAs land.
    eye = const.tile([128, 128], BF16)
    nc.sync.dma_start(eye[:], io["eye128"][:])
    diag = const.tile([128, 2, WLEN, 128], BF16)
    for w in range(2):
        for t in range(WLEN):
            nc.vector.tensor_scalar(
                out=diag[:, w, t], in0=eye[:],
                scalar1=Tb[:, w, t : t + 1], scalar2=None, op0=OP.mult,
            )
    FH = FLAT // 2  # 375, fits one PSUM bank in f32
    xt_scope = tc.tile_pool(name="xtp", bufs=1)
    xt_pool = xt_scope.__enter__()
    xt16 = xt_pool.tile([128, B_LOC, 2, FLAT], BF16, tag="xt16")
    with tc.tile_pool(name="psum_xt", bufs=2, space="PSUM") as pxt_pool, \
         tc.tile_pool(name="xfst", bufs=3) as xfst:
        for b in range(B_LOC):
            pts = {}
            for w in range(2):
                for half in range(2):
                    pts[w, half] = pxt_pool.tile(
                        [128, FH], F32, tag=f"xt{w}{half}", name=f"pxt{w}{half}"
                    )
            for t in range(WLEN):
                st = xfst.tile([128, FLAT], BF16, tag="xf")
                nc.sync.dma_start(
                    st[:],
                    x_full[b, t].rearrange("n i -> (n i)")
                    .rearrange("(p f) -> p f", p=128),
                )
                for w in range(2):
                    for half in range(2):
                        nc.tensor.matmul(
                            pts[w, half][:], diag[:, w, t],
                            st[:, half * FH : (half + 1) * FH],
                            start=(t == 0), stop=(t == WLEN - 1),
                        )
            for w in range(2):
                for half in range(2):
                    nc.scalar.copy(
                        xt16[:, b, w, half * FH : (half + 1) * FH], pts[w, half][:]
                    )

    # ================= A (pre phase PSUM) =================
    A = big.tile([128, NCHUNK, N], BF16, tag="A")
    rinv = const.tile([128, NCHUNK], F32)
    dsum_all = const.tile([128, NCHUNK], F32)

    with tc.tile_pool(name="prep", bufs=1) as prep:
        embT_raw = prep.tile([EMB, N], F32)
        nc.sync.dma_start(embT_raw[:], emb.rearrange("n d -> d n"))
        embT = prep.tile([EMB, N], F32R)
        nc.vector.tensor_copy(embT[:], embT_raw[:])
        with tc.tile_pool(name="psum_pre", bufs=2, space="PSUM") as psum_pre:
            for nch in range(NCHUNK):
                l = nlen(nch)
                nsl = slice(nch * 128, nch * 128 + l)
                pg = psum_pre.tile([128, N], F32, tag="pg")
                for mj in range(4):
                    m0 = mj * 512
                    mw = min(512, N - m0)
                    nc.tensor.matmul(
                        pg[:l, m0 : m0 + mw], embT[:, nsl],
                        embT[:, m0 : m0 + mw], start=True, stop=True,
                    )
                nc.scalar.activation(A[:l, nch, :], pg[:l, :], AF.Exp)
                nc.vector.tensor_scalar(
                    out=A[:l, nch, :], in0=A[:l, nch, :],
                    scalar1=1.0, scalar2=0.0, op0=OP.max, op1=OP.add,
                    accum_out=dsum_all[:l, nch : nch + 1],
                )
                nc.vector.reciprocal(rinv[:l, nch : nch + 1], dsum_all[:l, nch : nch + 1])

    # ================= x/state load; X1 [128, nch, b, 128] bf16 =================
    # SST keeps state resident for gating math (replaces per-chunk DMAs).
    # bf16: gpsimd-issued DMAs cast f32->bf16 in flight.
    SST = const.tile([128, NCHUNK, B_LOC, DOUT], BF16)
    X1 = big.tile([128, NCHUNK, B_LOC, 128], BF16, tag="slot1")
    nc.vector.memset(X1[:], 0.0)
    for b in range(B_LOC):
        xs = stage.tile([128, NCHUNK, DIN], F32, tag="stg")
        nc.vector.memset(xs[64:, NCHUNK - 1], 0.0)
        chunked_load(nc, xs, x[b])
        nc.vector.tensor_copy(X1[:, :, b, 0:DIN], xs[:])
        nc.vector.memset(SST[64:, NCHUNK - 1, b], 0.0)
        chunked_load(nc, SST[:, :, b], state[b], eng=nc.gpsimd)
        nc.scalar.copy(X1[:, :, b, DIN:CIN], SST[:, :, b])
    # ones col 96 meets WPg bias row 96 (k=0) -> gate z bias in PSUM
    nc.vector.memset(X1[:, :, :, 96:97], 1.0)

    # ================= window t-contraction (flat layout) =================
    # x_full[b, t] is accumulated in a flat [128, 750] view (2000*48 elems
    # row-major): elementwise sums don't care about layout, and flat DMAs are
    # fully contiguous. Results bounce through DRAM into packed [NPAD, 128]
    # transpose sources.
    dram = ctx.enter_context(tc.tile_pool(name="dram", bufs=6, space="DRAM"))
    HNCH = NCHUNK // 2   # half-panel: 8 n-chunks = 1024 rows

    def half_panel_T(SRC, b, h):
        """[128, 1024] <- transpose of SRC[:, h*8:(h+1)*8, b, :] via DRAM."""
        t = xgt_pool.tile([128, HNCH * 128], BF16, tag="xgt")
        dp = dram.tile([HNCH * 128, 128], BF16, tag="panh")
        nc.sync.dma_start(
            dp.rearrange("(c p) o -> p c o", p=128),
            SRC[:, h * HNCH : (h + 1) * HNCH, b, :],
        )
        nc.sync.dma_start(t[:], dp[:], transpose=True)
        return t

    # Materialized exactly: broadcast APs (stride-0 free dims) in DMAs leave
    # coverage holes on hardware -> undefined DRAM (NaN).
    zeros128 = const.tile([128, NCHUNK, 128], BF16)
    nc.vector.memset(zeros128[:], 0.0)
    dzero = dram.tile([NPAD, 128], BF16, tag="dzero")
    nc.sync.dma_start(dzero.rearrange("(c p) o -> p c o", p=128), zeros128[:])
    # ones column for XtT row 48 (bias folding: meets WWg/WWu bias rows).
    # Materialized exactly (no broadcast APs: a stride-0 mid free dim in a
    # DRAM->DRAM DMA lowers incorrectly).
    ones16 = const.tile([128, NCHUNK, 16], BF16)
    nc.vector.memset(ones16[:], 0.0)
    nc.vector.memset(ones16[:, :, 0:1], 1.0)
    dones = dram.tile([NPAD, 16], BF16, tag="dones")
    nc.sync.dma_start(dones.rearrange("(c p) o -> p c o", p=128), ones16[:])

    # pack via DRAM: XtT partitions 0:48 = xt_g.T, row 48 = ones, 64:112 = xt_u.T
    XtT = big.tile([128, R], BF16, tag="XtT")
    for b in range(B_LOC):
        dflat = dram.tile([2, 128, FLAT], BF16, tag="dflat")
        nc.sync.dma_start(dflat.rearrange("w p f -> p w f"), xt16[:, b])
        dpan = dram.tile([NPAD, 128], BF16, tag="pan")
        dfv = dflat.rearrange("w p f -> w (p f)").rearrange("w (n i) -> w n i", n=N)
        nc.sync.dma_start(dpan[0:N, 0:CW], dfv[0])
        nc.sync.dma_start(dpan[0:N, 64 : 64 + CW], dfv[1])
        # gap fill: ones at col 48 (bias row), zeros elsewhere
        nc.sync.dma_start(dpan[0:N, CW:64], dones[0:N])
        nc.sync.dma_start(dpan[0:N, 112:128], dzero[0:N, 0:16])
        nc.sync.dma_start(dpan[N:NPAD, :], dzero[N:NPAD, :])
        nc.sync.dma_start(XtT[:, b * NPAD : (b + 1) * NPAD], dpan[:], transpose=True)
        if DEBUG and b == 0:
            nc.sync.dma_start(io["dbg_dpan"][:], dpan[:])
    if DEBUG:
        nc.sync.dma_start(io["dbg_xtt"][:], XtT[:])
    xt_scope.__exit__(None, None, None)
    stage3_scope.__exit__(None, None, None)
    stage_scope.__exit__(None, None, None)

    # ================= diffusion helper =================
    def diffuse(psum_pool, SRC, DST, c0, clen):
        for nch in range(NCHUNK):
            l = nlen(nch)
            ph = psum_pool.tile([128, B_LOC, clen], F32, tag="pdiff")
            for mi in range(NCHUNK):
                ml = nlen(mi)
                nc.tensor.matmul(
                    ph[:l], A[:ml, mi, nch * 128 : nch * 128 + l],
                    SRC[:ml, mi, :, c0 : c0 + clen],
                    start=(mi == 0), stop=(mi == NCHUNK - 1),
                )
            nc.scalar.activation(
                DST[:l, nch, :, c0 : c0 + clen], ph[:l],
                AF.Copy, scale=rinv[:l, nch : nch + 1],
            )

    # ================= gate diffusion =================
    X2 = big.tile([128, NCHUNK, B_LOC, 128], BF16, tag="slot2")
    X3 = big.tile([128, NCHUNK, B_LOC, 128], BF16, tag="slot3")
    nc.gpsimd.memset(X2[:], 0.0)
    nc.gpsimd.memset(X3[:], 0.0)
    with tc.tile_pool(name="psum_d1", bufs=3, space="PSUM") as psum_d1:
        diffuse(psum_d1, X1, X2, 0, CIN)
        diffuse(psum_d1, X2, X3, 0, CIN)

    # ================= shared y-phase machinery =================
    acc_pool = ctx.enter_context(tc.tile_pool(name="accp", bufs=3))
    ybf_pool = ctx.enter_context(tc.tile_pool(name="ybf", bufs=2))
    xgt_pool = ctx.enter_context(tc.tile_pool(name="xgt", bufs=9))

    # eviction engines rotate to spread PSUM->SBUF traffic (Pool cannot
    # read PSUM on hardware).
    def _ev_act(dst, src):
        nc.scalar.copy(dst, src)

    def _ev_dve(dst, src):
        nc.vector.tensor_copy(dst, src)

    EV_PAT = [_ev_act, _ev_act, _ev_dve]

    def dred_group(yv, owid, nch):
        """In-place d-reduction on yv [128, 4, EMB, owid] (4 = b-pair x blk):
        scale block d by e[p, d] (tensor_scalar, 4x mode), then a pairwise
        in-place add tree over d (tensor_tensor, 2x mode). Result lands in
        yv[:, :, 0, :]; bias is already folded into the matmul (ones rows)."""
        for d in range(EMB):
            nc.vector.tensor_scalar(
                out=yv[:, :, d], in0=yv[:, :, d],
                scalar1=eexp[:, nch, d : d + 1], scalar2=None, op0=OP.mult,
            )
        step = 1
        while step < EMB:
            nc.vector.tensor_tensor(
                out=yv[:, :, 0 : EMB : 2 * step],
                in0=yv[:, :, 0 : EMB : 2 * step],
                in1=yv[:, :, step : EMB : 2 * step],
                op=OP.add,
            )
            step *= 2

    # ================= gate y-GEMM + d-red + gating =================
    r_gate = big.tile([128, NCHUNK, B_LOC, DOUT], BF16, tag="r_gate")

    ev_i = 0
    dr_i = 0
    with tc.tile_pool(name="psum_yg", bufs=2, space="PSUM") as psum_yg, \
         tc.tile_pool(name="psum_yw", bufs=2, space="PSUM") as psum_yw:
        for h in range(2):
          for bp in range(2):
            pair = (2 * bp, 2 * bp + 1)
            xgb = {b: [half_panel_T(S, b, h) for S in (X1, X2, X3)] for b in pair}
            for nch2 in range(HNCH):
                nch = h * HNCH + nch2
                l = nlen(nch)
                ybf2 = ybf_pool.tile([128, 2, 2, 1024], BF16, tag="ybf")
                for j, b in enumerate(pair):
                    r0 = b * NPAD + nch * 128
                    pg = psum_yg.tile([128, 1024], F32, tag="pyg")
                    for half in range(2):
                        for k in range(K):
                            nc.tensor.matmul(
                                pg[:, half * 512 : (half + 1) * 512],
                                xgb[b][k][:, nch2 * 128 : (nch2 + 1) * 128],
                                WPg[:, k, half * 512 : (half + 1) * 512],
                                start=(k == 0), stop=(k == K - 1),
                            )
                    pw = psum_yw.tile([128, 1024], F32, tag="pyw")
                    for half in range(2):
                        nc.tensor.matmul(
                            pw[:, half * 512 : (half + 1) * 512],
                            XtT[:, r0 : r0 + 128],
                            WWg[:, half * 512 : (half + 1) * 512],
                            start=True, stop=True,
                        )
                    EV_PAT[ev_i % len(EV_PAT)](ybf2[:, j, 0], pg[:])
                    EV_PAT[(ev_i + 1) % len(EV_PAT)](ybf2[:, j, 1], pw[:])
                    ev_i += 2
                    if DEBUG and h == 0 and bp == 0 and nch == 0 and j == 0:
                        nc.sync.dma_start(io["dbg"][:], ybf2[:, 0])
                yv = ybf2.rearrange("p b blk (d o) -> p (b blk) d o", d=EMB)
                dred_group(yv, DOUT, nch)
                for j, b in enumerate(pair):
                    ztile = acc_pool.tile([128, DOUT], BF16, tag="ztile")
                    nc.scalar.activation(ztile[:], ybf2[:, j, 0, 0:DOUT], AF.Sigmoid)
                    nc.scalar.activation(
                        r_gate[:, nch, b, :], ybf2[:, j, 1, 0:DOUT], AF.Sigmoid
                    )
                    # CAND panel reuses X1's slot: cols 0:32 keep x; stale cols
                    # 32:64 are neutralized by WPu's zero rows; z*state -> 64:128.
                    nc.gpsimd.tensor_mul(
                        X1[:, nch, b, 64:128], ztile[:], SST[:, nch, b]
                    )

    CAND = X1  # renamed: panels now hold [x | ones | stale | z*state]
    # ones col 32 meets WPu bias row 32 (k=0) -> update bias in PSUM (gate
    # y consumed the old state col 32 already; stale cols 33:64 are
    # neutralized by WPu's zero rows).
    nc.vector.memset(X1[:, :, :, 32:33], 1.0)

    # ================= update diffusion =================
    # C2/C3 reuse X2/X3 slots: cols 0:32 already hold diffused-x hops; zero
    # 32:64; diffusion writes 64:128.
    C2, C3 = X2, X3
    with tc.tile_pool(name="psum_d2", bufs=3, space="PSUM") as psum_d2:
        diffuse(psum_d2, CAND, C2, 64, DOUT)
        diffuse(psum_d2, C2, C3, 64, DOUT)

    # ================= update y-GEMM + d-red + output =================
    with tc.tile_pool(name="psum_yu", bufs=3, space="PSUM") as psum_yu, \
         tc.tile_pool(name="psum_uw", bufs=3, space="PSUM") as psum_uw:
        for h in range(2):
          for bp in range(2):
            pair = (2 * bp, 2 * bp + 1)
            xgb = {b: [half_panel_T(S, b, h) for S in (CAND, C2, C3)] for b in pair}
            for nch2 in range(HNCH):
                nch = h * HNCH + nch2
                l = nlen(nch)
                ubf_full = ybf_pool.tile([128, 2, 2, 1024], BF16, tag="ybf")
                ubf2 = ubf_full[:, :, :, 0:512]
                for j, b in enumerate(pair):
                    r0 = b * NPAD + nch * 128
                    pu = psum_yu.tile([128, 512], F32, tag="pyu")
                    for k in range(K):
                        nc.tensor.matmul(
                            pu[:], xgb[b][k][:, nch2 * 128 : (nch2 + 1) * 128],
                            start=(k == 0), stop=(k == K - 1), rhs=WPu[:, k, :],
                        )
                    uw = psum_uw.tile([128, 512], F32, tag="puw")
                    nc.tensor.matmul(
                        uw[:], XtT[:, r0 : r0 + 128], WWu[:],
                        start=True, stop=True,
                    )
                    EV_PAT[ev_i % len(EV_PAT)](ubf2[:, j, 0], pu[:])
                    EV_PAT[(ev_i + 1) % len(EV_PAT)](ubf2[:, j, 1], uw[:])
                    ev_i += 2
                uv = ubf2.rearrange("p b blk (d o) -> p (b blk) d o", d=EMB)
                dred_group(uv, 32, nch)
                tmp2 = acc_pool.tile([128, 2, DOUT], F32, tag="tmp2")
                for j, b in enumerate(pair):
                    hc = acc_pool.tile([128, DOUT], F32, tag="hc")
                    nc.scalar.activation(
                        hc.rearrange("p (blk o) -> p blk o", blk=2),
                        ubf2[:, j, :, 0:32], AF.Tanh,
                    )
                    nc.gpsimd.tensor_sub(tmp2[:, j], SST[:, nch, b], hc[:])
                    nc.gpsimd.tensor_mul(tmp2[:, j], tmp2[:, j], r_gate[:, nch, b, :])
                    nc.gpsimd.tensor_add(tmp2[:, j], tmp2[:, j], hc[:])
                nc.sync.dma_start(
                    out[pair[0] : pair[0] + 2, nch * 128 : nch * 128 + l, :]
                    .rearrange("b n o -> n b o"),
                    tmp2[:l],
                )


def make_io(nc):
    io = {}
    io["x"] = nc.dram_tensor("x", [B_LOC, N, DIN], F32, kind="ExternalInput")
    io["state"] = nc.dram_tensor("state", [B_LOC, N, DOUT], F32, kind="ExternalInput")
    io["x_full"] = nc.dram_tensor("x_full", [B_LOC, WLEN, N, CW], BF16, kind="ExternalInput")
    io["eye128"] = nc.dram_tensor("eye128", [128, 128], BF16, kind="ExternalInput")
    io["node_embeddings"] = nc.dram_tensor("node_embeddings", [N, EMB], F32, kind="ExternalInput")
    io["gw_pool"] = nc.dram_tensor("gw_pool", [EMB, K, CIN, 64], F32, kind="ExternalInput")
    io["gw_win"] = nc.dram_tensor("gw_win", [EMB, CW, 64], F32, kind="ExternalInput")
    io["gb_pool"] = nc.dram_tensor("gb_pool", [EMB, 2 * DOUT], F32, kind="ExternalInput")
    io["gT"] = nc.dram_tensor("gT", [WLEN], F32, kind="ExternalInput")
    io["uw_pool"] = nc.dram_tensor("uw_pool", [EMB, K, CIN, 32], F32, kind="ExternalInput")
    io["uw_win"] = nc.dram_tensor("uw_win", [EMB, CW, 32], F32, kind="ExternalInput")
    io["ub_pool"] = nc.dram_tensor("ub_pool", [EMB, DOUT], F32, kind="ExternalInput")
    io["uT"] = nc.dram_tensor("uT", [WLEN], F32, kind="ExternalInput")
    io["out"] = nc.dram_tensor("out", [B_LOC, N, DOUT], F32, kind="ExternalOutput")
    if DEBUG:
        io["dbg"] = nc.dram_tensor("dbg", [128, 2, EMB * 64], BF16, kind="ExternalOutput")
        io["dbg_dpan"] = nc.dram_tensor("dbg_dpan", [NPAD, 128], BF16, kind="ExternalOutput")
        io["dbg_xtt"] = nc.dram_tensor("dbg_xtt", [128, R], BF16, kind="ExternalOutput")
    return io


def build_module(debug=False):
    from concourse import bacc

    nc = bacc.Bacc("TRN2", target_bir_lowering=False, debug=debug)
    io = make_io(nc)
    with tile.TileContext(nc) as tc:
        build(tc, io)
    nc.finalize()
    return nc


# ======================= harness wrapper =======================
import numpy as _np

N_CORES = 8
_CACHE = {}


def _get_module():
    if "nc" not in _CACHE:
        _CACHE["nc"] = build_module()
    return _CACHE["nc"]


def kernel(**inputs):
    """Full-input entry point: shards over batch across 8 NeuronCores."""
    import ml_dtypes

    nc = _get_module()
    from concourse.bass_utils import run_bass_kernel_spmd

    bf16 = ml_dtypes.bfloat16
    xb = _np.ascontiguousarray(inputs["x"], dtype=_np.float32)
    sb = _np.ascontiguousarray(inputs["state"], dtype=_np.float32)
    xf = _np.ascontiguousarray(
        _np.asarray(inputs["x_full"], dtype=_np.float32).astype(bf16)
    )
    rep = {
        k: _np.ascontiguousarray(inputs[k], dtype=_np.float32)
        for k in ("node_embeddings", "gw_pool", "gw_win", "gb_pool", "gT",
                  "uw_pool", "uw_win", "ub_pool", "uT")
    }
    rep["eye128"] = _np.eye(128, dtype=bf16)
    in_maps = []
    for i in range(N_CORES):
        m = dict(rep)
        m["x"] = xb[i * B_LOC : (i + 1) * B_LOC]
        m["state"] = sb[i * B_LOC : (i + 1) * B_LOC]
        m["x_full"] = xf[i * B_LOC : (i + 1) * B_LOC]
        in_maps.append(m)
    res = run_bass_kernel_spmd(nc, in_maps, core_ids=list(range(N_CORES)))
    return _np.concatenate([res.results[i]["out"] for i in range(N_CORES)], axis=0)



# revision 52
# speedup vs baseline: 1.1879x; 1.0006x over previous
import sys
for _p in ('/opt/trn_rl_repo',):
    if _p not in sys.path:
        sys.path.insert(0, _p)

"""NLSGCRN cell Bass/Tile kernel for TRN2, batch-sharded SPMD over 8 cores.

Per-core shapes (b_loc = 4 batches):
  x [4,2000,32], state [4,2000,64], x_full [4,12,2000,48], emb [2000,16],
  pools gw/uw/gb/ub/gT/uT, out h [4,2000,64].

Strategy:
  A = exp(relu(emb emb^T)) (symmetric, bf16), d = rowsum, rinv = 1/d;
  s^k x = A-chunk matmuls with 1/d row-scale on PSUM eviction (natural
  [n, (b,c)] layout). Per-node grouped GEMM via the D-expansion:
  y[r,(d,o)] = Xg[r,ki] @ WP[ki,(d,o)], z[r,o] = bias + sum_d e[r,d]*y[r,(d,o)]
  with per-partition-scalar fused multiply-add (scalar_tensor_tensor) split
  DVE (direct from PSUM) / GPSIMD (after bf16 eviction by ACT).
  Window conv: xt = sum_t T[t] x_full[:,t] accumulated on GPSIMD, contracted
  with the same machinery (ki = 48).
"""

from contextlib import ExitStack

import concourse.bass as bass
import concourse.tile as tile
from concourse import mybir
from concourse._compat import with_exitstack

F32 = mybir.dt.float32
F32R = mybir.dt.float32r
BF16 = mybir.dt.bfloat16
AF = mybir.ActivationFunctionType
OP = mybir.AluOpType

B_LOC = 4
N = 2000
NCHUNK = 16           # ceil(2000/128)
NFULL = (NCHUNK - 1) * 128   # 1920
NPAD = NCHUNK * 128   # 2048
R = B_LOC * NPAD      # 8192 padded rows
NRC = R // 128        # 64 row-chunks
DIN, DOUT = 32, 64
CIN = 96
CW = 48
WLEN = 12
EMB = 16
K = 3

FLAT = N * CW // 128  # 750 (flat per-partition window elements)
DEBUG = False


def nlen(nch):
    return 128 if nch < NCHUNK - 1 else N - NFULL  # last = 80


def chunked_load(nc, dst, src, eng=None):
    """dst [128, NCHUNK, ...inner] <- src [2000, ...inner] splitting rows."""
    eng = eng or nc.sync
    inner = src.shape[1:]
    eng.dma_start(
        dst[:, 0 : NCHUNK - 1],
        src[0:NFULL].rearrange(
            "(c p) " + " ".join(f"i{j}" for j in range(len(inner)))
            + " -> p c " + " ".join(f"i{j}" for j in range(len(inner))),
            p=128,
        ),
    )
    eng.dma_start(dst[0 : N - NFULL, NCHUNK - 1], src[NFULL:N])


@with_exitstack
def build(ctx: ExitStack, tc: tile.TileContext, io: dict):
    nc = tc.nc

    io = {k: (v[:] if not isinstance(v, bass.AP) else v) for k, v in io.items()}
    x, state, x_full = io["x"], io["state"], io["x_full"]
    emb = io["node_embeddings"]
    out = io["out"]

    const = ctx.enter_context(tc.tile_pool(name="const", bufs=1))
    big = ctx.enter_context(tc.tile_pool(name="big", bufs=1))
    stage_scope = tc.tile_pool(name="stage", bufs=3)
    stage = stage_scope.__enter__()
    stage3_scope = tc.tile_pool(name="stage3", bufs=2)
    stage3 = stage3_scope.__enter__()

    # ================= constants / weights =================
    eexp = const.tile([128, NCHUNK, EMB], F32)
    nc.vector.memset(eexp[:], 0.0)
    chunked_load(nc, eexp, emb)

    # WPg [128, 3, 1024] bf16: rows 0:96 = c, cols (d,o) d-major.
    # Pad rows of the weight tiles can be anything on HW (the matching XgT
    # partitions are zero), but memset keeps the simulator's init-tracking
    # happy.
    WPg = const.tile([128, K, EMB * 64], BF16)
    nc.vector.memset(WPg[:], 0.0)
    WPu = const.tile([128, K, EMB * 32], BF16)
    nc.vector.memset(WPu[:], 0.0)
    WWg = const.tile([128, EMB * 64], BF16)
    nc.vector.memset(WWg[:], 0.0)
    WWu = const.tile([128, EMB * 32], BF16)
    nc.vector.memset(WWu[:], 0.0)
    for k in range(K):
        wk = stage.tile([128, EMB, 64], F32, tag="stg")
        nc.sync.dma_start(wk[0:CIN], io["gw_pool"][:, k].rearrange("d c o -> c d o"))
        nc.vector.tensor_copy(
            WPg[0:CIN, k].rearrange("p (d o) -> p d o", d=EMB), wk[0:CIN]
        )
        wku = stage.tile([128, EMB, 32], F32, tag="stg")
        nc.sync.dma_start(wku[0:32], io["uw_pool"][:, k, 0:32, :].rearrange("d c o -> c d o"))
        nc.sync.dma_start(wku[64:128], io["uw_pool"][:, k, 32:96, :].rearrange("d c o -> c d o"))
        nc.vector.tensor_copy(
            WPu[0:32, k].rearrange("p (d o) -> p d o", d=EMB), wku[0:32]
        )
        nc.vector.tensor_copy(
            WPu[64:128, k].rearrange("p (d o) -> p d o", d=EMB), wku[64:128]
        )
    wg = stage.tile([128, EMB, 64], F32, tag="stg")
    nc.sync.dma_start(wg[0:CW], io["gw_win"].rearrange("d i o -> i d o"))
    nc.vector.tensor_copy(WWg[0:CW].rearrange("p (d o) -> p d o", d=EMB), wg[0:CW])
    wu = stage.tile([128, EMB, 32], F32, tag="stg")
    # rows 64:112 (matches packed XtT where xt_u.T sits at partitions 64:112)
    nc.sync.dma_start(wu[64 : 64 + CW], io["uw_win"].rearrange("d i o -> i d o"))
    nc.vector.tensor_copy(
        WWu[64 : 64 + CW].rearrange("p (d o) -> p d o", d=EMB), wu[64 : 64 + CW]
    )

    # Bias folding: ones-rows in the X panels / XtT meet these weight rows, so
    # PSUM accumulates emb@bias_pool without a separate bias pass.
    #   gate z bias -> WPg row 96 (X1 ones col 96, k=0)
    #   gate r bias -> WWg row 48 (XtT ones row 48)
    #   update graph bias -> WPu row 32 (CAND ones col 32, k=0)
    #   update window bias -> WWu row 48
    # gpsimd DMAs cast f32 -> bf16 in flight.
    def _row1(src2d):
        return bass.AP(tensor=src2d.tensor, offset=src2d.offset,
                       ap=[[0, 1]] + [list(d) for d in src2d.ap])

    nc.gpsimd.dma_start(
        WPg[96:97, 0, :].rearrange("p (d o) -> p d o", d=EMB),
        _row1(io["gb_pool"][:, 0:DOUT]),
    )
    nc.gpsimd.dma_start(
        WWg[48:49, :].rearrange("p (d o) -> p d o", d=EMB),
        _row1(io["gb_pool"][:, DOUT:]),
    )
    nc.gpsimd.dma_start(
        WPu[32:33, 0, :].rearrange("p (d o) -> p d o", d=EMB),
        _row1(io["ub_pool"][:, 0:32]),
    )
    nc.gpsimd.dma_start(
        WWu[48:49, :].rearrange("p (d o) -> p d o", d=EMB),
        _row1(io["ub_pool"][:, 32:64]),
    )

    Tb = const.tile([128, 2, WLEN], F32)
    for w, name in ((0, "gT"), (1, "uT")):
        src = io[name][:]
        nc.sync.dma_start(
            Tb[:, w, :],
            bass.AP(tensor=src.tensor, offset=src.offset, ap=[[0, 128]] + list(src.ap)),
        )

    # ================= window t-contraction on PE =================
    # xt[p, f] = sum_t T[t] * xf_t[p, f] as 12 accumulating matmuls with
    # stationary diag(T[t]) built from the host-provided identity. Runs first
    # so PE ramps up while embT/x/state DMAs land.
    eye = const.tile([128, 128], BF16)
    nc.sync.dma_start(eye[:], io["eye128"][:])
    diag = const.tile([128, 2, WLEN, 128], BF16)
    for w in range(2):
        for t in range(WLEN):
            nc.vector.tensor_scalar(
                out=diag[:, w, t], in0=eye[:],
                scalar1=Tb[:, w, t : t + 1], scalar2=None, op0=OP.mult,
            )
    FH = FLAT // 2  # 375, fits one PSUM bank in f32
    xt_scope = tc.tile_pool(name="xtp", bufs=1)
    xt_pool = xt_scope.__enter__()
    xt16 = xt_pool.tile([128, B_LOC, 2, FLAT], BF16, tag="xt16")
    with tc.tile_pool(name="psum_xt", bufs=2, space="PSUM") as pxt_pool, \
         tc.tile_pool(name="xfst", bufs=3) as xfst:
        for b in range(B_LOC):
            pts = {}
            for w in range(2):
                for half in range(2):
                    pts[w, half] = pxt_pool.tile(
                        [128, FH], F32, tag=f"xt{w}{half}", name=f"pxt{w}{half}"
                    )
            for tg in range(WLEN // 4):
                st = xfst.tile([128, 4, FLAT], BF16, tag="xf")
                nc.sync.dma_start(
                    st[:],
                    x_full[b, 4 * tg : 4 * tg + 4]
                    .rearrange("t n i -> t (n i)")
                    .rearrange("t (p f) -> p t f", p=128),
                )
                for tt in range(4):
                    t = 4 * tg + tt
                    for w in range(2):
                        for half in range(2):
                            nc.tensor.matmul(
                                pts[w, half][:], diag[:, w, t],
                                st[:, tt, half * FH : (half + 1) * FH],
                                start=(t == 0), stop=(t == WLEN - 1),
                            )
            for w in range(2):
                for half in range(2):
                    nc.scalar.copy(
                        xt16[:, b, w, half * FH : (half + 1) * FH], pts[w, half][:]
                    )

    # ================= A (pre phase PSUM) =================
    A = big.tile([128, NCHUNK, N], BF16, tag="A")
    rinv = const.tile([128, NCHUNK], F32)
    dsum_all = const.tile([128, NCHUNK], F32)

    with tc.tile_pool(name="prep", bufs=1) as prep:
        embT_raw = prep.tile([EMB, N], F32)
        nc.sync.dma_start(embT_raw[:], emb.rearrange("n d -> d n"))
        embT = prep.tile([EMB, N], F32R)
        nc.vector.tensor_copy(embT[:], embT_raw[:])
        with tc.tile_pool(name="psum_pre", bufs=2, space="PSUM") as psum_pre:
            for nch in range(NCHUNK):
                l = nlen(nch)
                nsl = slice(nch * 128, nch * 128 + l)
                pg = psum_pre.tile([128, N], F32, tag="pg")
                for mj in range(4):
                    m0 = mj * 512
                    mw = min(512, N - m0)
                    nc.tensor.matmul(
                        pg[:l, m0 : m0 + mw], embT[:, nsl],
                        embT[:, m0 : m0 + mw], start=True, stop=True,
                    )
                nc.scalar.activation(A[:l, nch, :], pg[:l, :], AF.Exp)
                nc.vector.tensor_scalar(
                    out=A[:l, nch, :], in0=A[:l, nch, :],
                    scalar1=1.0, scalar2=0.0, op0=OP.max, op1=OP.add,
                    accum_out=dsum_all[:l, nch : nch + 1],
                )
                nc.vector.reciprocal(rinv[:l, nch : nch + 1], dsum_all[:l, nch : nch + 1])

    # ================= x/state load; X1 [128, nch, b, 128] bf16 =================
    # SST keeps state resident for gating math (replaces per-chunk DMAs).
    # bf16: gpsimd-issued DMAs cast f32->bf16 in flight.
    SST = const.tile([128, NCHUNK, B_LOC, DOUT], BF16)
    X1 = big.tile([128, NCHUNK, B_LOC, 128], BF16, tag="slot1")
    nc.vector.memset(X1[:], 0.0)
    for b in range(B_LOC):
        xs = stage.tile([128, NCHUNK, DIN], F32, tag="stg")
        nc.vector.memset(xs[64:, NCHUNK - 1], 0.0)
        chunked_load(nc, xs, x[b])
        nc.vector.tensor_copy(X1[:, :, b, 0:DIN], xs[:])
        nc.vector.memset(SST[64:, NCHUNK - 1, b], 0.0)
        chunked_load(nc, SST[:, :, b], state[b], eng=nc.gpsimd)
        nc.scalar.copy(X1[:, :, b, DIN:CIN], SST[:, :, b])
    # ones col 96 meets WPg bias row 96 (k=0) -> gate z bias in PSUM
    nc.vector.memset(X1[:, :, :, 96:97], 1.0)

    # ================= window t-contraction (flat layout) =================
    # x_full[b, t] is accumulated in a flat [128, 750] view (2000*48 elems
    # row-major): elementwise sums don't care about layout, and flat DMAs are
    # fully contiguous. Results bounce through DRAM into packed [NPAD, 128]
    # transpose sources.
    dram = ctx.enter_context(tc.tile_pool(name="dram", bufs=6, space="DRAM"))
    HNCH = NCHUNK // 2   # half-panel: 8 n-chunks = 1024 rows

    def pair_panel_T(SRC, bp, h):
        """[128, 2, 1024] <- transposes of SRC[:, h*8:(h+1)*8, b, :] for the
        b-pair (2bp, 2bp+1), one DRAM bounce + one transpose for both."""
        t = xgt_pool.tile([128, 2 * HNCH * 128], BF16, tag="xgt")
        dp = dram.tile([2 * HNCH * 128, 128], BF16, tag="panh")
        dpv = dp.rearrange("(b c p) o -> p b c o", p=128, b=2)
        for j in range(2):
            nc.gpsimd.dma_start(
                dpv[:, j],
                SRC[:, h * HNCH : (h + 1) * HNCH, 2 * bp + j, :],
            )
        nc.sync.dma_start(t[:], dp[:], transpose=True)
        return t.rearrange("p (b f) -> p b f", b=2)

    # Materialized exactly: broadcast APs (stride-0 free dims) in DMAs leave
    # coverage holes on hardware -> undefined DRAM (NaN).
    zeros128 = const.tile([128, NCHUNK, 128], BF16)
    nc.vector.memset(zeros128[:], 0.0)
    dzero = dram.tile([NPAD, 128], BF16, tag="dzero")
    nc.sync.dma_start(dzero.rearrange("(c p) o -> p c o", p=128), zeros128[:])
    # ones column for XtT row 48 (bias folding: meets WWg/WWu bias rows).
    # Materialized exactly (no broadcast APs: a stride-0 mid free dim in a
    # DRAM->DRAM DMA lowers incorrectly).
    ones16 = const.tile([128, NCHUNK, 16], BF16)
    nc.vector.memset(ones16[:], 0.0)
    nc.vector.memset(ones16[:, :, 0:1], 1.0)
    dones = dram.tile([NPAD, 16], BF16, tag="dones")
    nc.sync.dma_start(dones.rearrange("(c p) o -> p c o", p=128), ones16[:])

    # pack via DRAM: XtT partitions 0:48 = xt_g.T, row 48 = ones, 64:112 = xt_u.T
    # dpan is a single persistent buffer: constant regions (ones col, zero
    # gaps, pad rows) are filled once; per-b only the data columns rewrite.
    XtT = big.tile([128, R], BF16, tag="XtT")
    dpan = dram.tile([NPAD, 128], BF16, tag="pan")
    nc.gpsimd.dma_start(dpan[0:N, CW:64], dones[0:N])
    nc.gpsimd.dma_start(dpan[0:N, 112:128], dzero[0:N, 0:16])
    nc.gpsimd.dma_start(dpan[N:NPAD, :], dzero[N:NPAD, :])
    for b in range(B_LOC):
        dflat = dram.tile([2, 128, FLAT], BF16, tag="dflat")
        nc.gpsimd.dma_start(dflat.rearrange("w p f -> p w f"), xt16[:, b])
        dfv = dflat.rearrange("w p f -> w (p f)").rearrange("w (n i) -> w n i", n=N)
        nc.gpsimd.dma_start(dpan[0:N, 0:CW], dfv[0])
        nc.gpsimd.dma_start(dpan[0:N, 64 : 64 + CW], dfv[1])
        nc.sync.dma_start(XtT[:, b * NPAD : (b + 1) * NPAD], dpan[:], transpose=True)
        if DEBUG and b == 0:
            nc.sync.dma_start(io["dbg_dpan"][:], dpan[:])
    if DEBUG:
        nc.sync.dma_start(io["dbg_xtt"][:], XtT[:])
    xt_scope.__exit__(None, None, None)
    stage3_scope.__exit__(None, None, None)
    stage_scope.__exit__(None, None, None)

    # ================= diffusion helper =================
    def diffuse(psum_pool, SRC, DST, c0, clen):
        for nch in range(NCHUNK):
            l = nlen(nch)
            ph = psum_pool.tile([128, B_LOC, clen], F32, tag="pdiff")
            for mi in range(NCHUNK):
                ml = nlen(mi)
                nc.tensor.matmul(
                    ph[:l], A[:ml, mi, nch * 128 : nch * 128 + l],
                    SRC[:ml, mi, :, c0 : c0 + clen],
                    start=(mi == 0), stop=(mi == NCHUNK - 1),
                )
            nc.scalar.activation(
                DST[:l, nch, :, c0 : c0 + clen], ph[:l],
                AF.Copy, scale=rinv[:l, nch : nch + 1],
            )

    # ================= gate diffusion =================
    X2 = big.tile([128, NCHUNK, B_LOC, 128], BF16, tag="slot2")
    X3 = big.tile([128, NCHUNK, B_LOC, 128], BF16, tag="slot3")
    nc.gpsimd.memset(X2[:], 0.0)
    nc.gpsimd.memset(X3[:], 0.0)
    with tc.tile_pool(name="psum_d1", bufs=3, space="PSUM") as psum_d1:
        diffuse(psum_d1, X1, X2, 0, CIN)
        diffuse(psum_d1, X2, X3, 0, CIN)

    # ================= shared y-phase machinery =================
    acc_pool = ctx.enter_context(tc.tile_pool(name="accp", bufs=3))
    ybf_pool = ctx.enter_context(tc.tile_pool(name="ybf", bufs=2))
    xgt_pool = ctx.enter_context(tc.tile_pool(name="xgt", bufs=5))

    # eviction engines rotate to spread PSUM->SBUF traffic (Pool cannot
    # read PSUM on hardware).
    def _ev_act(dst, src):
        nc.scalar.copy(dst, src)

    def _ev_dve(dst, src):
        nc.vector.tensor_copy(dst, src)

    EV_PAT = [_ev_act, _ev_act, _ev_dve]

    def dred_group(yv, owid, nch):
        """In-place d-reduction on yv [128, 4, EMB, owid] (4 = b-pair x blk):
        scale block d by e[p, d] (tensor_scalar, 4x mode), then a pairwise
        in-place add tree over d (tensor_tensor, 2x mode). Result lands in
        yv[:, :, 0, :]; bias is already folded into the matmul (ones rows)."""
        for d in range(EMB):
            nc.vector.tensor_scalar(
                out=yv[:, :, d], in0=yv[:, :, d],
                scalar1=eexp[:, nch, d : d + 1], scalar2=None, op0=OP.mult,
            )
        step = 1
        while step < EMB:
            nc.vector.tensor_tensor(
                out=yv[:, :, 0 : EMB : 2 * step],
                in0=yv[:, :, 0 : EMB : 2 * step],
                in1=yv[:, :, step : EMB : 2 * step],
                op=OP.add,
            )
            step *= 2

    # ================= gate y-GEMM + d-red + gating =================
    r_gate = big.tile([128, NCHUNK, B_LOC, DOUT], BF16, tag="r_gate")

    ev_i = 0
    dr_i = 0
    with tc.tile_pool(name="psum_yg", bufs=2, space="PSUM") as psum_yg, \
         tc.tile_pool(name="psum_yw", bufs=2, space="PSUM") as psum_yw:
        for h in range(2):
          for bp in range(2):
            pair = (2 * bp, 2 * bp + 1)
            xgb = [pair_panel_T(S, bp, h) for S in (X1, X2, X3)]
            for nch2 in range(HNCH):
                nch = h * HNCH + nch2
                l = nlen(nch)
                ybf2 = ybf_pool.tile([128, 2, 2, 1024], BF16, tag="ybf")
                for j, b in enumerate(pair):
                    r0 = b * NPAD + nch * 128
                    pg = psum_yg.tile([128, 1024], F32, tag="pyg")
                    for half in range(2):
                        for k in range(K):
                            nc.tensor.matmul(
                                pg[:, half * 512 : (half + 1) * 512],
                                xgb[k][:, j, nch2 * 128 : (nch2 + 1) * 128],
                                WPg[:, k, half * 512 : (half + 1) * 512],
                                start=(k == 0), stop=(k == K - 1),
                            )
                    pw = psum_yw.tile([128, 1024], F32, tag="pyw")
                    for half in range(2):
                        nc.tensor.matmul(
                            pw[:, half * 512 : (half + 1) * 512],
                            XtT[:, r0 : r0 + 128],
                            WWg[:, half * 512 : (half + 1) * 512],
                            start=True, stop=True,
                        )
                    EV_PAT[ev_i % len(EV_PAT)](ybf2[:, j, 0], pg[:])
                    EV_PAT[(ev_i + 1) % len(EV_PAT)](ybf2[:, j, 1], pw[:])
                    ev_i += 2
                    if DEBUG and h == 0 and bp == 0 and nch == 0 and j == 0:
                        nc.sync.dma_start(io["dbg"][:], ybf2[:, 0])
                yv = ybf2.rearrange("p b blk (d o) -> p (b blk) d o", d=EMB)
                dred_group(yv, DOUT, nch)
                for j, b in enumerate(pair):
                    ztile = acc_pool.tile([128, DOUT], BF16, tag="ztile")
                    nc.scalar.activation(ztile[:], ybf2[:, j, 0, 0:DOUT], AF.Sigmoid)
                    nc.scalar.activation(
                        r_gate[:, nch, b, :], ybf2[:, j, 1, 0:DOUT], AF.Sigmoid
                    )
                    # CAND panel reuses X1's slot: cols 0:32 keep x; stale cols
                    # 32:64 are neutralized by WPu's zero rows; z*state -> 64:128.
                    nc.gpsimd.tensor_mul(
                        X1[:, nch, b, 64:128], ztile[:], SST[:, nch, b]
                    )

    CAND = X1  # renamed: panels now hold [x | ones | stale | z*state]
    # ones col 32 meets WPu bias row 32 (k=0) -> update bias in PSUM (gate
    # y consumed the old state col 32 already; stale cols 33:64 are
    # neutralized by WPu's zero rows).
    nc.vector.memset(X1[:, :, :, 32:33], 1.0)

    # ================= update diffusion =================
    # C2/C3 reuse X2/X3 slots: cols 0:32 already hold diffused-x hops; zero
    # 32:64; diffusion writes 64:128.
    C2, C3 = X2, X3
    with tc.tile_pool(name="psum_d2", bufs=3, space="PSUM") as psum_d2:
        diffuse(psum_d2, CAND, C2, 64, DOUT)
        diffuse(psum_d2, C2, C3, 64, DOUT)

    # ================= update y-GEMM + d-red + output =================
    with tc.tile_pool(name="psum_yu", bufs=3, space="PSUM") as psum_yu, \
         tc.tile_pool(name="psum_uw", bufs=3, space="PSUM") as psum_uw:
        for h in range(2):
          for bp in range(2):
            pair = (2 * bp, 2 * bp + 1)
            xgb = [pair_panel_T(S, bp, h) for S in (CAND, C2, C3)]
            for nch2 in range(HNCH):
                nch = h * HNCH + nch2
                l = nlen(nch)
                ubf_full = ybf_pool.tile([128, 2, 2, 1024], BF16, tag="ybf")
                ubf2 = ubf_full[:, :, :, 0:512]
                for j, b in enumerate(pair):
                    r0 = b * NPAD + nch * 128
                    pu = psum_yu.tile([128, 512], F32, tag="pyu")
                    for k in range(K):
                        nc.tensor.matmul(
                            pu[:], xgb[k][:, j, nch2 * 128 : (nch2 + 1) * 128],
                            start=(k == 0), stop=(k == K - 1), rhs=WPu[:, k, :],
                        )
                    uw = psum_uw.tile([128, 512], F32, tag="puw")
                    nc.tensor.matmul(
                        uw[:], XtT[:, r0 : r0 + 128], WWu[:],
                        start=True, stop=True,
                    )
                    EV_PAT[ev_i % len(EV_PAT)](ubf2[:, j, 0], pu[:])
                    EV_PAT[(ev_i + 1) % len(EV_PAT)](ubf2[:, j, 1], uw[:])
                    ev_i += 2
                uv = ubf2.rearrange("p b blk (d o) -> p (b blk) d o", d=EMB)
                dred_group(uv, 32, nch)
                tmp2 = acc_pool.tile([128, 2, DOUT], F32, tag="tmp2")
                for j, b in enumerate(pair):
                    hc = acc_pool.tile([128, DOUT], F32, tag="hc")
                    nc.scalar.activation(
                        hc.rearrange("p (blk o) -> p blk o", blk=2),
                        ubf2[:, j, :, 0:32], AF.Tanh,
                    )
                    nc.gpsimd.tensor_sub(tmp2[:, j], SST[:, nch, b], hc[:])
                    nc.gpsimd.tensor_mul(tmp2[:, j], tmp2[:, j], r_gate[:, nch, b, :])
                    nc.gpsimd.tensor_add(tmp2[:, j], tmp2[:, j], hc[:])
                nc.gpsimd.dma_start(
                    out[pair[0] : pair[0] + 2, nch * 128 : nch * 128 + l, :]
                    .rearrange("b n o -> n b o"),
                    tmp2[:l],
                )


def make_io(nc):
    io = {}
    io["x"] = nc.dram_tensor("x", [B_LOC, N, DIN], F32, kind="ExternalInput")
    io["state"] = nc.dram_tensor("state", [B_LOC, N, DOUT], F32, kind="ExternalInput")
    io["x_full"] = nc.dram_tensor("x_full", [B_LOC, WLEN, N, CW], BF16, kind="ExternalInput")
    io["eye128"] = nc.dram_tensor("eye128", [128, 128], BF16, kind="ExternalInput")
    io["node_embeddings"] = nc.dram_tensor("node_embeddings", [N, EMB], F32, kind="ExternalInput")
    io["gw_pool"] = nc.dram_tensor("gw_pool", [EMB, K, CIN, 64], F32, kind="ExternalInput")
    io["gw_win"] = nc.dram_tensor("gw_win", [EMB, CW, 64], F32, kind="ExternalInput")
    io["gb_pool"] = nc.dram_tensor("gb_pool", [EMB, 2 * DOUT], F32, kind="ExternalInput")
    io["gT"] = nc.dram_tensor("gT", [WLEN], F32, kind="ExternalInput")
    io["uw_pool"] = nc.dram_tensor("uw_pool", [EMB, K, CIN, 32], F32, kind="ExternalInput")
    io["uw_win"] = nc.dram_tensor("uw_win", [EMB, CW, 32], F32, kind="ExternalInput")
    io["ub_pool"] = nc.dram_tensor("ub_pool", [EMB, DOUT], F32, kind="ExternalInput")
    io["uT"] = nc.dram_tensor("uT", [WLEN], F32, kind="ExternalInput")
    io["out"] = nc.dram_tensor("out", [B_LOC, N, DOUT], F32, kind="ExternalOutput")
    if DEBUG:
        io["dbg"] = nc.dram_tensor("dbg", [128, 2, EMB * 64], BF16, kind="ExternalOutput")
        io["dbg_dpan"] = nc.dram_tensor("dbg_dpan", [NPAD, 128], BF16, kind="ExternalOutput")
        io["dbg_xtt"] = nc.dram_tensor("dbg_xtt", [128, R], BF16, kind="ExternalOutput")
    return io


def build_module(debug=False):
    from concourse import bacc

    nc = bacc.Bacc("TRN2", target_bir_lowering=False, debug=debug)
    io = make_io(nc)
    with tile.TileContext(nc) as tc:
        build(tc, io)
    nc.finalize()
    return nc


# ======================= harness wrapper =======================
import numpy as _np

N_CORES = 8
_CACHE = {}


def _get_module():
    if "nc" not in _CACHE:
        _CACHE["nc"] = build_module()
    return _CACHE["nc"]


def kernel(**inputs):
    """Full-input entry point: shards over batch across 8 NeuronCores."""
    import ml_dtypes

    nc = _get_module()
    from concourse.bass_utils import run_bass_kernel_spmd

    bf16 = ml_dtypes.bfloat16
    xb = _np.ascontiguousarray(inputs["x"], dtype=_np.float32)
    sb = _np.ascontiguousarray(inputs["state"], dtype=_np.float32)
    xf = _np.ascontiguousarray(
        _np.asarray(inputs["x_full"], dtype=_np.float32).astype(bf16)
    )
    rep = {
        k: _np.ascontiguousarray(inputs[k], dtype=_np.float32)
        for k in ("node_embeddings", "gw_pool", "gw_win", "gb_pool", "gT",
                  "uw_pool", "uw_win", "ub_pool", "uT")
    }
    rep["eye128"] = _np.eye(128, dtype=bf16)
    in_maps = []
    for i in range(N_CORES):
        m = dict(rep)
        m["x"] = xb[i * B_LOC : (i + 1) * B_LOC]
        m["state"] = sb[i * B_LOC : (i + 1) * B_LOC]
        m["x_full"] = xf[i * B_LOC : (i + 1) * B_LOC]
        in_maps.append(m)
    res = run_bass_kernel_spmd(nc, in_maps, core_ids=list(range(N_CORES)))
    return _np.concatenate([res.results[i]["out"] for i in range(N_CORES)], axis=0)

